# revision 3
# baseline (speedup 1.0000x reference)
import numpy as np

# nn_NewsEncoder: hardcoded problem shapes
VOCAB, D_MODEL, SEQ, H, HD, ATT = 50000, 300, 30, 20, 20, 200
B = 4096
N_CORES = 8
SHARD = B // N_CORES  # 512 — pure data parallel over batch


def _forward_shard_np(x, emb, pe, Wq, bq, Wv, bv, Wa, ba, wq2, bq2):
    # x: [b, S] int -> z: [b, H*HD] f32  (exact numpy fallback path)
    b = x.shape[0]
    e = emb[x] + pe[None, :, :]
    e_flat = e.reshape(b * SEQ, D_MODEL)

    Wq_stack = Wq.reshape(H * D_MODEL, D_MODEL)
    q = e_flat @ Wq_stack.T
    q = q.reshape(b, SEQ, H, D_MODEL).transpose(0, 2, 1, 3)
    q = q + bq[None, :, None, :]

    eT = e.transpose(0, 2, 1)[:, None, :, :]
    scores = np.matmul(q, eT)

    m = scores.max(axis=-2, keepdims=True)
    ex = np.exp(scores - m)
    attn = ex / ex.sum(axis=-2, keepdims=True)

    v = np.matmul(e[:, None, :, :], Wv.transpose(0, 2, 1)[None])
    hv = np.matmul(attn, v) + bv[None, :, None, :]

    h = hv.transpose(0, 2, 1, 3).reshape(b, SEQ, H * HD)

    a = np.tanh(h.reshape(b * SEQ, H * HD) @ Wa.T + ba) @ wq2.T + bq2
    a = a.reshape(b, SEQ)
    z = np.einsum("bs,bsd->bd", a, h)
    return z.astype(np.float32)


def _kernel_np(x, emb, pe, Wq, bq, Wv, bv, Wa, ba, wq2, bq2):
    outs = []
    for c in range(N_CORES):
        xs = x[c * SHARD : (c + 1) * SHARD]
        outs.append(_forward_shard_np(xs, emb, pe, Wq, bq, Wv, bv, Wa, ba, wq2, bq2))
    return np.concatenate(outs, axis=0)


def _kernel_trn(x, emb, pe, Wq, bq, Wv, bv, Wa, ba, wq2, bq2):
    # Data-parallel across the 8 NeuronCores: batch dim of x sharded 8 ways,
    # weights + embedding table replicated (per sharding hint). The whole
    # forward runs on-device via XLA/neuronx-cc.
    import jax
    import jax.numpy as jnp

    devs = jax.devices()
    if len(devs) < N_CORES:
        raise RuntimeError(f"need {N_CORES} cores, have {len(devs)}")

    def fwd(xs, emb, pe, Wq, bq, Wv, bv, Wa, ba, wq2, bq2):
        b = xs.shape[0]
        e = emb[xs] + pe[None]                                   # [b,S,D]
        ef = e.reshape(b * SEQ, D_MODEL)
        q = (ef @ Wq.reshape(H * D_MODEL, D_MODEL).T).reshape(b, SEQ, H, D_MODEL)
        q = q.transpose(0, 2, 1, 3) + bq[None, :, None, :]       # [b,H,S,D]
        scores = jnp.einsum("bhso,bto->bhst", q, e)              # [b,H,S,S]
        m = scores.max(axis=-2, keepdims=True)
        ex = jnp.exp(scores - m)
        attn = ex / ex.sum(axis=-2, keepdims=True)
        v = jnp.einsum("btd,hod->bhto", e, Wv)
        hv = jnp.matmul(attn, v) + bv[None, :, None, :]          # [b,H,S,HD]
        h = hv.transpose(0, 2, 1, 3).reshape(b, SEQ, H * HD)
        a = jnp.tanh(h.reshape(b * SEQ, H * HD) @ Wa.T + ba) @ wq2.T + bq2
        z = jnp.einsum("bs,bsd->bd", a.reshape(b, SEQ), h)
        return z

    pfwd = jax.pmap(fwd, in_axes=(0,) + (None,) * 10)
    xs = np.ascontiguousarray(x.astype(np.int32).reshape(N_CORES, SHARD, SEQ))
    out = pfwd(xs, emb, pe, Wq, bq, Wv, bv, Wa, ba, wq2, bq2)
    return np.asarray(out).reshape(B, H * HD).astype(np.float32)


def kernel(x, emb, pe, Wq, bq, Wv, bv, Wa, ba, wq2, bq2):
    x = np.asarray(x)
    emb = np.asarray(emb, dtype=np.float32)
    pe = np.asarray(pe, dtype=np.float32)
    Wq = np.asarray(Wq, dtype=np.float32)
    bq = np.asarray(bq, dtype=np.float32)
    Wv = np.asarray(Wv, dtype=np.float32)
    bv = np.asarray(bv, dtype=np.float32)
    Wa = np.asarray(Wa, dtype=np.float32)
    ba = np.asarray(ba, dtype=np.float32)
    wq2 = np.asarray(wq2, dtype=np.float32)
    bq2 = np.asarray(bq2, dtype=np.float32)

    try:
        return _kernel_trn(x, emb, pe, Wq, bq, Wv, bv, Wa, ba, wq2, bq2)
    except Exception:
        return _kernel_np(x, emb, pe, Wq, bq, Wv, bv, Wa, ba, wq2, bq2)
